# revision 25
# baseline (speedup 1.0000x reference)
"""Trainium2 Bass kernel for CausalSelfAttention (GQA + RoPE + sliding window).

Module: B=2, S=2048, E=2048, NH=16 heads, NKV=4 kv heads, HD=128,
WINDOW=1024 (local causal: 0 <= q-k < 1024), fp32 in/out.

Sharding (8 cores): core = b*4 + g  where b = batch (2), g = kv-head group (4).
Each core handles 1 batch x 1 kv head (4 q heads), computes a partial
out-projection with its Wo column block; the host sums the 4 partials per
batch (the "all-reduce" of the TP sharding done at unshard time).

All device data is bf16 (PSUM accumulation stays fp32): halves HBM traffic,
host<->device staging, and DVE element work vs an fp32 version, at ~3e-3
relative error (budget 2e-2).

Layouts on device (all transposed, feature-on-partition):
  xT   [E, S]      (input, transposed on host, bf16)
  qT/kT[128, S]    per head chunk, RoPE applied during PSUM eviction
  v    [S, 128]    natural (via PE transposes) for the PV matmul
  scoresT [k,q]    so softmax denominator = ones-matmul, PV needs no transpose
  yT   [E, S]      partial output, bf16 (host converts, transposes + sums)

Attention tiling: q chunks of 512. Fully-in-window k-tiles ([k=128] each) are
processed 512-wide in pairs; partially-masked k-tiles are split into 256-wide
q halves — fully-masked halves are skipped, fully-valid halves need no mask,
the rest multiply by a 0/1 mask slice after exp.

build_nc(repeat=R) emits the whole per-call body R times back-to-back in one
NEFF; timing harnesses use (t(R) - t(1)) / (R - 1) to measure per-execution
device time independent of host dispatch overhead.
"""

import math

import numpy as np

B, S, E = 2, 2048, 2048
NH, NKV, HD = 16, 4, 128
WINDOW = 1024
P = 128
QC = 512  # q chunk (moving free dim)
HC = 256  # half chunk for partial tiles
N_QC = S // QC  # 4
N_E = E // P  # 16 contraction chunks
SCALE = 1.0 / math.sqrt(HD)

# mask deltas: delta = q0 - 128*kt for partially-masked [k=128, q] tiles.
# 256-wide masks are column slices [:, :256] of the same patterns.
MASK_DELTAS = [-384, -256, -128, 0, 640, 768, 896, 1024]
MASK_IDX = {d: i for i, d in enumerate(MASK_DELTAS)}


def _kt_range(qc):
    kt_lo = max(0, (qc * QC - (WINDOW - 1)) // P)
    kt_hi = (qc * QC + QC - 1) // P
    return list(range(kt_lo, kt_hi + 1))


def _full_partial(qc):
    """Split k-tiles for q chunk qc into 512-wide full tiles and 256-wide
    partial units. Returns (full_kts, units) where units = [(kt, h2, mask_delta
    or None)] and fully-masked halves are dropped."""
    full, units = [], []
    for kt in _kt_range(qc):
        d = QC * qc - P * kt
        if 128 <= d <= 512:
            full.append(kt)
            continue
        for h2 in range(2):
            dh = d + h2 * HC
            lo, hi = dh - (P - 1), dh + (HC - 1)  # dist range in this half
            if hi < 0 or lo >= WINDOW:
                continue  # fully masked
            if lo >= 0 and hi < WINDOW:
                units.append((kt, h2, None))  # fully valid
            else:
                assert dh in MASK_IDX, (qc, kt, h2, dh)
                units.append((kt, h2, dh))
    return full, units


def build_nc(repeat=1, loop_n=0, staggered=False):
    """loop_n > 0 wraps one body in a hardware For_i loop executing it
    loop_n times (used by the timing harness to amortize host dispatch)."""
    import concourse.bass as bass
    import concourse.mybir as mybir
    import concourse.tile as tile
    from concourse import bacc
    from concourse.masks import make_identity

    f32 = mybir.dt.float32
    bf16 = mybir.dt.bfloat16
    Exp = mybir.ActivationFunctionType.Exp

    nc = bacc.Bacc("TRN2", target_bir_lowering=False, debug=False, num_devices=8)

    xT = nc.dram_tensor("xT", [E, S], bf16, kind="ExternalInput")
    # wqkvT: [E, 768] = concat(WqT_g [E,512], WkT_g [E,128], WvT_g [E,128])
    wqkvT = nc.dram_tensor("wqkvT", [E, 768], bf16, kind="ExternalInput")
    # woT_g: [512, E] = Wo[:, g*512:(g+1)*512].T
    woT = nc.dram_tensor("woT", [4 * P, E], bf16, kind="ExternalInput")
    cosT = nc.dram_tensor("cosT", [P, S], bf16, kind="ExternalInput")
    sinFT = nc.dram_tensor("sinFT", [P, S], bf16, kind="ExternalInput")
    y = nc.dram_tensor("y", [E, S], bf16, kind="ExternalOutput")  # yT layout

    with tile.TileContext(nc) as tc:
        with tc.tile_pool(name="persist", bufs=1) as pp:
            # persistent SBUF tensors (shared/overwritten across reps)
            qT_r = [pp.tile([P, S], bf16, tag=f"qT{h}", name=f"qT{h}") for h in range(4)]
            kT_r = pp.tile([P, S], bf16, tag="kT", name="kT")
            v_nat = pp.tile([P, S], bf16, tag="v_nat", name="v_nat")  # [k%128, kt*128+d]
            ident = pp.tile([P, P], bf16, tag="ident", name="ident")
            make_identity(nc, ident[:])
            ones_col_f = pp.tile([P, 1], f32, tag="ones_col_f", name="ones_col_f")
            ones_col = pp.tile([P, 1], bf16, tag="ones_col", name="ones_col")
            nc.vector.memset(ones_col_f[:], 1.0)
            nc.vector.tensor_copy(ones_col[:], ones_col_f[:])

            if loop_n > 0:
                hint = (
                    mybir.EngineType.PE,
                    mybir.EngineType.Activation,
                    mybir.EngineType.DVE,
                    mybir.EngineType.SP,
                    mybir.EngineType.Pool,
                )
                with tc.For_i(0, loop_n, 1, hint_engines=hint, staggered_reset=staggered):
                    _emit_body(
                        nc, tc, 0, qT_r, kT_r, v_nat, ident, ones_col,
                        xT, wqkvT, woT, cosT, sinFT, y, f32, bf16, Exp,
                    )
            else:
                for rep in range(repeat):
                    _emit_body(
                        nc, tc, rep, qT_r, kT_r, v_nat, ident, ones_col,
                        xT, wqkvT, woT, cosT, sinFT, y, f32, bf16, Exp,
                    )

    nc.compile()
    return nc


def _emit_body(
    nc, tc, rep, qT_r, kT_r, v_nat, ident, ones_col,
    xT, wqkvT, woT, cosT, sinFT, y, f32, bf16, Exp,
):
    import concourse.tile as tile  # noqa: F401

    R = f"r{rep}_"

    # ---------------- Phase 1: QKV projections + RoPE + v transpose
    with (
        tc.tile_pool(name=R + "wqkv_pool", bufs=1) as wqp,
        tc.tile_pool(name=R + "xpool", bufs=3) as xp,
        tc.tile_pool(name=R + "cspool", bufs=2) as csp,
        tc.tile_pool(name=R + "vstage", bufs=3) as vsp,
        tc.tile_pool(name=R + "proj_ps", bufs=1, space="PSUM") as pps,
        tc.tile_pool(name=R + "vtr_ps", bufs=1, space="PSUM") as vtps,
    ):
        wqkv_r = []
        x_pre = {}
        for e in range(N_E):
            t = wqp.tile([P, 768], bf16, tag=f"wqkv{e}", name=f"{R}wqkv{e}")
            nc.sync.dma_start(out=t[:], in_=wqkvT[e * P:(e + 1) * P, :])
            wqkv_r.append(t)
            # interleave s=0 x tiles 1:1 with weight DMAs, on the
            # second HWDGE queue (Activation), matching consumption
            # order so PE is never input-starved.
            x_r0 = xp.tile([P, QC], bf16, tag="x_r", bufs=6, name=f"{R}x_r0_{e}")
            nc.scalar.dma_start(out=x_r0[:], in_=xT[e * P:(e + 1) * P, 0:QC])
            x_pre[(0, e)] = x_r0

        cos_all = csp.tile([P, S], bf16, tag="cos_all", bufs=1, name=R + "cos_all")
        sinF_all = csp.tile([P, S], bf16, tag="sinF_all", bufs=1, name=R + "sinF_all")
        nc.scalar.dma_start(out=cos_all[:], in_=cosT[:])
        nc.scalar.dma_start(out=sinF_all[:], in_=sinFT[:])

        for s in range(N_QC):
            ssl = slice(s * QC, (s + 1) * QC)
            cos_sb = cos_all[:, ssl]
            sinF_sb = sinF_all[:, ssl]

            ps = [
                pps.tile([P, QC], f32, tag=f"proj{(f + s) % 7}", name=f"{R}proj{f}_{s}")
                for f in range(6)
            ]
            for e in range(N_E):
                if (s, e) in x_pre:
                    x_r = x_pre[(s, e)]
                else:
                    x_r = xp.tile(
                        [P, QC], bf16, tag="x_r", bufs=6, name=f"{R}x_r{s}_{e}"
                    )
                    nc.scalar.dma_start(out=x_r[:], in_=xT[e * P:(e + 1) * P, ssl])
                for f in range(6):
                    nc.tensor.matmul(
                        ps[f][:],
                        wqkv_r[e][:, f * P:(f + 1) * P],
                        x_r[:],
                        start=(e == 0),
                        stop=(e == N_E - 1),
                    )

            # evict psum fast via ACT copy (frees the bank), then
            # RoPE on SBUF off the PSUM critical path:
            # dst = stage*cos + shift(stage)*sinF
            def rope_evict(dst, psrc, tmp_name):
                stage = xp.tile([P, QC], bf16, tag="rstage", bufs=4, name="st" + tmp_name)
                nc.scalar.copy(stage[:], psrc)
                # partition-rotate by 64 via single-input copies
                # (SBUF TT requires equal base partitions on HW)
                shf = xp.tile([P, QC], bf16, tag="rope_shf", name="sh" + tmp_name)
                H = P // 2
                nc.vector.tensor_copy(shf[0:H, :], stage[H:P, :])
                nc.vector.tensor_copy(shf[H:P, :], stage[0:H, :])
                nc.vector.tensor_mul(shf[:], shf[:], sinF_sb)
                nc.vector.tensor_mul(stage[:], stage[:], cos_sb)
                nc.vector.tensor_add(dst, stage[:], shf[:])

            rope_evict(kT_r[:, ssl], ps[4][:], f"{R}rope_k{s}")

            # v: evict bf16, then PE-transpose each 128 block
            v_sb = vsp.tile([P, QC], bf16, tag="v_sb", name=f"{R}v_sb{s}")
            nc.scalar.copy(v_sb[:], ps[5][:])
            for j in range(QC // P):
                kt = s * (QC // P) + j
                tps = vtps.tile([P, P], bf16, tag="vtr", name=f"{R}vtr{kt}")
                nc.tensor.transpose(tps[:], v_sb[:, j * P:(j + 1) * P], ident[:])
                nc.vector.tensor_copy(v_nat[:, kt * P:(kt + 1) * P], tps[:])

            for h in range(4):
                rope_evict(qT_r[h][:, ssl], ps[h][:], f"{R}rope_q{h}_{s}")

    # ---------------- Phase 2+3: attention + out-projection
    with (
        tc.tile_pool(name=R + "wo_pool", bufs=1) as wop,
        tc.tile_pool(name=R + "exp_pool", bufs=6) as ep,
        tc.tile_pool(name=R + "outT_pool", bufs=1) as op_,
        tc.tile_pool(name=R + "small_pool", bufs=3) as sp,
        tc.tile_pool(name=R + "sc_ps", bufs=2, space="PSUM") as scp,
        tc.tile_pool(name=R + "pv_ps", bufs=1, space="PSUM") as pvp,
        tc.tile_pool(name=R + "denbc_ps", bufs=1, space="PSUM") as dbp,
        tc.tile_pool(name=R + "yp_ps", bufs=2, space="PSUM") as ypp,
    ):
        # Wo resident load (needed first by oproj(qc0), after attn(qc0))
        wo_r = []
        for d in range(4):
            t = wop.tile([P, E], bf16, tag=f"wo_r{d}", name=f"{R}wo_r{d}")
            nc.sync.dma_start(out=t[:], in_=woT[d * P:(d + 1) * P, :])
            wo_r.append(t)

        outT = [
            op_.tile([P, S], bf16, tag=f"outT{h}", name=f"{R}outT{h}")
            for h in range(4)
        ]

        for qc in range(N_QC):
            qsl = slice(qc * QC, (qc + 1) * QC)
            full_kts, units = _full_partial(qc)
            for h in range(4):
                pv = pvp.tile([P, QC], f32, tag="pv", name=f"{R}pv{qc}_{h}")
                den = dbp.tile([1, QC], f32, tag="denbc", name=f"{R}den{qc}_{h}")

                # PSUM accumulate flags: start=True on the first
                # matmul into the bank zeroes the whole 2KB zero
                # region, so later matmuls accumulate start=False
                # into either q-half; stop=True only on the last.
                ops = []  # (kind, payload)
                for i in range(0, len(full_kts), 2):
                    ops.append(("full_pair", full_kts[i:i + 2]))
                for i in range(0, len(units), 4):
                    ops.append(("unit_quad", units[i:i + 4]))
                n_acc = sum(len(pl) for _, pl in ops)

                def acc_flags(oid_):
                    return oid_ == 0, oid_ == n_acc - 1

                oid = 0
                for kind, pl in ops:
                    if kind == "full_pair":
                        pair = pl
                        w = QC
                        sc = scp.tile(
                            [P, 2 * QC], f32, tag="sc",
                            name=f"{R}sc{qc}_{h}_{pair[0]}",
                        )
                        for j, kt in enumerate(pair):
                            nc.tensor.matmul(
                                sc[:, j * w:(j + 1) * w],
                                kT_r[:, kt * P:(kt + 1) * P],
                                qT_r[h][:, qsl],
                                start=True,
                                stop=True,
                            )
                        ex = ep.tile(
                            [P, 2 * QC], bf16, tag="ex",
                            name=f"{R}ex{qc}_{h}_f{pair[0]}",
                        )
                        nc.scalar.activation(
                            ex[:, : len(pair) * w],
                            sc[:, : len(pair) * w],
                            Exp,
                            scale=SCALE,
                        )
                        for j, kt in enumerate(pair):
                            exj = ex[:, j * w:(j + 1) * w]
                            st, sp_ = acc_flags(oid)
                            nc.tensor.matmul(
                                pv[:],
                                v_nat[:, kt * P:(kt + 1) * P],
                                exj,
                                start=st,
                                stop=sp_,
                            )
                            nc.tensor.matmul(
                                den[:],
                                ones_col[:],
                                exj,
                                start=st,
                                stop=sp_,
                            )
                            oid += 1
                    else:
                        upair = pl
                        w = HC
                        sc = scp.tile(
                            [P, 2 * QC], f32, tag="sc",
                            name=f"{R}scu{qc}_{h}_{upair[0][0]}_{upair[0][1]}",
                        )  # holds up to 4 x 256-wide units
                        for j, (kt, h2, dh) in enumerate(upair):
                            q0 = qc * QC + h2 * HC
                            nc.tensor.matmul(
                                sc[:, j * w:(j + 1) * w],
                                kT_r[:, kt * P:(kt + 1) * P],
                                qT_r[h][:, q0:q0 + HC],
                                start=True,
                                stop=True,
                            )
                        ex = ep.tile(
                            [P, 2 * QC], bf16, tag="ex",
                            name=f"{R}exu{qc}_{h}_{upair[0][0]}_{upair[0][1]}",
                        )
                        nc.scalar.activation(
                            ex[:, : len(upair) * w],
                            sc[:, : len(upair) * w],
                            Exp,
                            scale=SCALE,
                        )
                        for j, (kt, h2, dh) in enumerate(upair):
                            exj = ex[:, j * w:(j + 1) * w]
                            if dh is not None:
                                # zero out-of-window (iota = dh + q - k):
                                # dh <= 0 cuts dist >= 0; dh >= 640 cuts
                                # dist < WINDOW. exactly one cut per tile.
                                import concourse.mybir as _mybir
                                if dh <= 0:
                                    nc.gpsimd.affine_select(
                                        out=exj, in_=exj,
                                        pattern=[[1, HC]],
                                        compare_op=_mybir.AluOpType.is_ge,
                                        fill=0.0, base=dh,
                                        channel_multiplier=-1,
                                    )
                                else:
                                    # keep where (W-1-dh) + k - q >= 0
                                    nc.gpsimd.affine_select(
                                        out=exj, in_=exj,
                                        pattern=[[-1, HC]],
                                        compare_op=_mybir.AluOpType.is_ge,
                                        fill=0.0, base=(WINDOW - 1) - dh,
                                        channel_multiplier=1,
                                    )
                            st, sp_ = acc_flags(oid)
                            pv_reg = pv[:, h2 * HC:(h2 + 1) * HC]
                            den_reg = den[:, h2 * HC:(h2 + 1) * HC]
                            nc.tensor.matmul(
                                pv_reg,
                                v_nat[:, kt * P:(kt + 1) * P],
                                exj,
                                start=st,
                                stop=sp_,
                            )
                            nc.tensor.matmul(
                                den_reg,
                                ones_col[:],
                                exj,
                                start=st,
                                stop=sp_,
                            )
                            oid += 1

                # normalize: outT[h][:, qsl] = pv * (1/den) broadcast
                recip = sp.tile([1, QC], f32, tag="recip", name=f"{R}rc{qc}_{h}")
                nc.vector.reciprocal(recip[:], den[:])
                bc_sb = sp.tile([P, QC], f32, tag="bc_sb", name=f"{R}bcs{qc}_{h}")
                nc.gpsimd.partition_broadcast(bc_sb[:], recip[:])
                nc.vector.tensor_mul(outT[h][:, qsl], pv[:], bc_sb[:])

            # out-projection for this q chunk (own psum pool)
            for e in range(N_E):
                yp = ypp.tile([P, QC], f32, tag="yp", name=f"{R}yp{qc}_{e}")
                for d in range(4):
                    nc.tensor.matmul(
                        yp[:],
                        wo_r[d][:, e * P:(e + 1) * P],
                        outT[d][:, qsl],
                        start=(d == 0),
                        stop=(d == 3),
                    )
                y_sb = sp.tile([P, QC], bf16, tag="y_sb", bufs=6, name=f"{R}ysb{qc}_{e}")
                nc.vector.tensor_copy(y_sb[:], yp[:])
                nc.sync.dma_start(out=y[e * P:(e + 1) * P, qsl], in_=y_sb[:])


def _bf16(a):
    import ml_dtypes

    return np.ascontiguousarray(a).astype(ml_dtypes.bfloat16)


def make_in_maps(x, cos, sin, Wq, Wk, Wv, Wo):
    cosT = np.ascontiguousarray(cos[:, 0, :].T)  # [128, S]
    sinT = sin[:, 0, :].T
    sinFT = np.concatenate([-sinT[: HD // 2], sinT[HD // 2:]], axis=0)
    sinFT = np.ascontiguousarray(sinFT.astype(np.float32))
    cosT_b = _bf16(cosT)
    sinFT_b = _bf16(sinFT)
    in_maps = []
    for c in range(8):
        b, g = c // 4, c % 4
        wq_g = Wq[g * 4 * HD:(g + 1) * 4 * HD, :]  # [512, E]
        wk_g = Wk[g * HD:(g + 1) * HD, :]  # [128, E]
        wv_g = Wv[g * HD:(g + 1) * HD, :]
        wqkvT = np.ascontiguousarray(
            np.concatenate([wq_g, wk_g, wv_g], axis=0).T
        )  # [E, 768]
        woT_g = np.ascontiguousarray(Wo[:, g * 4 * HD:(g + 1) * 4 * HD].T)  # [512, E]
        in_maps.append(
            {
                "xT": _bf16(x[b].T),
                "wqkvT": _bf16(wqkvT),
                "woT": _bf16(woT_g),
                "cosT": cosT_b,
                "sinFT": sinFT_b,
            }
        )
    return in_maps


_NC_CACHE = {}


def get_nc():
    if "nc" not in _NC_CACHE:
        _NC_CACHE["nc"] = build_nc()
    return _NC_CACHE["nc"]


def kernel(x, cos, sin, Wq, Wk, Wv, Wo):
    from concourse.bass_utils import run_bass_kernel_spmd

    x = np.asarray(x, dtype=np.float32)
    cos = np.asarray(cos, dtype=np.float32)
    sin = np.asarray(sin, dtype=np.float32)
    Wq = np.asarray(Wq, dtype=np.float32)
    Wk = np.asarray(Wk, dtype=np.float32)
    Wv = np.asarray(Wv, dtype=np.float32)
    Wo = np.asarray(Wo, dtype=np.float32)

    nc = get_nc()
    in_maps = make_in_maps(x, cos, sin, Wq, Wk, Wv, Wo)
    res = run_bass_kernel_spmd(nc, in_maps, core_ids=list(range(8)))
    out = np.zeros((B, S, E), dtype=np.float32)
    for c in range(8):
        b = c // 4
        out[b] += res.results[c]["y"].T.astype(np.float32)
    return out


# revision 26
# speedup vs baseline: 1.0136x; 1.0136x over previous
"""Trainium2 Bass kernel for CausalSelfAttention (GQA + RoPE + sliding window).

Module: B=2, S=2048, E=2048, NH=16 heads, NKV=4 kv heads, HD=128,
WINDOW=1024 (local causal: 0 <= q-k < 1024), fp32 in/out.

Sharding (8 cores): core = b*4 + g  where b = batch (2), g = kv-head group (4).
Each core handles 1 batch x 1 kv head (4 q heads), computes a partial
out-projection with its Wo column block; the host sums the 4 partials per
batch (the "all-reduce" of the TP sharding done at unshard time).

All device data is bf16 (PSUM accumulation stays fp32): halves HBM traffic,
host<->device staging, and DVE element work vs an fp32 version, at ~3e-3
relative error (budget 2e-2).

Layouts on device (all transposed, feature-on-partition):
  xT   [E, S]      (input, transposed on host, bf16)
  qT/kT[128, S]    per head chunk, RoPE applied during PSUM eviction
  v    [S, 128]    natural (via PE transposes) for the PV matmul
  scoresT [k,q]    so softmax denominator = ones-matmul, PV needs no transpose
  yT   [E, S]      partial output, bf16 (host converts, transposes + sums)

Attention tiling: q chunks of 512. Fully-in-window k-tiles ([k=128] each) are
processed 512-wide in pairs; partially-masked k-tiles are split into 256-wide
q halves — fully-masked halves are skipped, fully-valid halves need no mask,
the rest multiply by a 0/1 mask slice after exp.

build_nc(repeat=R) emits the whole per-call body R times back-to-back in one
NEFF; timing harnesses use (t(R) - t(1)) / (R - 1) to measure per-execution
device time independent of host dispatch overhead.
"""

import math

import numpy as np

B, S, E = 2, 2048, 2048
NH, NKV, HD = 16, 4, 128
WINDOW = 1024
P = 128
QC = 512  # q chunk (moving free dim)
HC = 256  # half chunk for partial tiles
N_QC = S // QC  # 4
N_E = E // P  # 16 contraction chunks
SCALE = 1.0 / math.sqrt(HD)

# mask deltas: delta = q0 - 128*kt for partially-masked [k=128, q] tiles.
# 256-wide masks are column slices [:, :256] of the same patterns.
MASK_DELTAS = [-384, -256, -128, 0, 640, 768, 896, 1024]
MASK_IDX = {d: i for i, d in enumerate(MASK_DELTAS)}


def _kt_range(qc):
    kt_lo = max(0, (qc * QC - (WINDOW - 1)) // P)
    kt_hi = (qc * QC + QC - 1) // P
    return list(range(kt_lo, kt_hi + 1))


def _full_partial(qc):
    """Split k-tiles for q chunk qc into 512-wide full tiles and 256-wide
    partial units. Returns (full_kts, units) where units = [(kt, h2, mask_delta
    or None)] and fully-masked halves are dropped."""
    full, units = [], []
    for kt in _kt_range(qc):
        d = QC * qc - P * kt
        if 128 <= d <= 512:
            full.append(kt)
            continue
        for h2 in range(2):
            dh = d + h2 * HC
            lo, hi = dh - (P - 1), dh + (HC - 1)  # dist range in this half
            if hi < 0 or lo >= WINDOW:
                continue  # fully masked
            if lo >= 0 and hi < WINDOW:
                units.append((kt, h2, None))  # fully valid
            else:
                assert dh in MASK_IDX, (qc, kt, h2, dh)
                units.append((kt, h2, dh))
    return full, units


def build_nc(repeat=1, loop_n=0, staggered=False):
    """loop_n > 0 wraps one body in a hardware For_i loop executing it
    loop_n times (used by the timing harness to amortize host dispatch)."""
    import concourse.bass as bass
    import concourse.mybir as mybir
    import concourse.tile as tile
    from concourse import bacc
    from concourse.masks import make_identity

    f32 = mybir.dt.float32
    bf16 = mybir.dt.bfloat16
    Exp = mybir.ActivationFunctionType.Exp

    nc = bacc.Bacc("TRN2", target_bir_lowering=False, debug=False, num_devices=8)

    xT = nc.dram_tensor("xT", [E, S], bf16, kind="ExternalInput")
    # wqkvT: [E, 768] = concat(WqT_g [E,512], WkT_g [E,128], WvT_g [E,128])
    wqkvT = nc.dram_tensor("wqkvT", [E, 768], bf16, kind="ExternalInput")
    # woT_g: [512, E] = Wo[:, g*512:(g+1)*512].T
    woT = nc.dram_tensor("woT", [4 * P, E], bf16, kind="ExternalInput")
    cosT = nc.dram_tensor("cosT", [P, S], bf16, kind="ExternalInput")
    sinFT = nc.dram_tensor("sinFT", [P, S], bf16, kind="ExternalInput")
    y = nc.dram_tensor("y", [E, S], bf16, kind="ExternalOutput")  # yT layout

    with tile.TileContext(nc) as tc:
        with tc.tile_pool(name="persist", bufs=1) as pp:
            # persistent SBUF tensors (shared/overwritten across reps)
            qT_r = [pp.tile([P, S], bf16, tag=f"qT{h}", name=f"qT{h}") for h in range(4)]
            kT_r = pp.tile([P, S], bf16, tag="kT", name="kT")
            v_nat = pp.tile([P, S], bf16, tag="v_nat", name="v_nat")  # [k%128, kt*128+d]
            ident = pp.tile([P, P], bf16, tag="ident", name="ident")
            make_identity(nc, ident[:])
            ones_col_f = pp.tile([P, 1], f32, tag="ones_col_f", name="ones_col_f")
            ones_col = pp.tile([P, 1], bf16, tag="ones_col", name="ones_col")
            nc.vector.memset(ones_col_f[:], 1.0)
            nc.vector.tensor_copy(ones_col[:], ones_col_f[:])

            if loop_n > 0:
                hint = (
                    mybir.EngineType.PE,
                    mybir.EngineType.Activation,
                    mybir.EngineType.DVE,
                    mybir.EngineType.SP,
                    mybir.EngineType.Pool,
                )
                with tc.For_i(0, loop_n, 1, hint_engines=hint, staggered_reset=staggered):
                    _emit_body(
                        nc, tc, 0, qT_r, kT_r, v_nat, ident, ones_col,
                        xT, wqkvT, woT, cosT, sinFT, y, f32, bf16, Exp,
                    )
            else:
                for rep in range(repeat):
                    _emit_body(
                        nc, tc, rep, qT_r, kT_r, v_nat, ident, ones_col,
                        xT, wqkvT, woT, cosT, sinFT, y, f32, bf16, Exp,
                    )

    nc.compile()
    return nc


def _emit_body(
    nc, tc, rep, qT_r, kT_r, v_nat, ident, ones_col,
    xT, wqkvT, woT, cosT, sinFT, y, f32, bf16, Exp,
):
    import concourse.tile as tile  # noqa: F401

    R = f"r{rep}_"

    # ---------------- Phase 1: QKV projections + RoPE + v transpose
    with (
        tc.tile_pool(name=R + "wqkv_pool", bufs=1) as wqp,
        tc.tile_pool(name=R + "xpool", bufs=3) as xp,
        tc.tile_pool(name=R + "cspool", bufs=2) as csp,
        tc.tile_pool(name=R + "vstage", bufs=3) as vsp,
        tc.tile_pool(name=R + "proj_ps", bufs=1, space="PSUM") as pps,
        tc.tile_pool(name=R + "vtr_ps", bufs=1, space="PSUM") as vtps,
    ):
        wqkv_r = []
        x_pre = {}
        for e in range(N_E):
            t = wqp.tile([P, 768], bf16, tag=f"wqkv{e}", name=f"{R}wqkv{e}")
            nc.sync.dma_start(out=t[:], in_=wqkvT[e * P:(e + 1) * P, :])
            wqkv_r.append(t)
            # interleave x super-chunk tiles (2 s-chunks wide: 2KB/partition
            # DMA lines, the HWDGE efficiency knee) 1:1 with weight DMAs on
            # the second HWDGE queue so PE is never input-starved.
            x_r0 = xp.tile([P, 2 * QC], bf16, tag="x_r", bufs=18, name=f"{R}x_r0_{e}")
            nc.scalar.dma_start(out=x_r0[:], in_=xT[e * P:(e + 1) * P, 0:2 * QC])
            x_pre[(0, e)] = x_r0

        cos_all = csp.tile([P, S], bf16, tag="cos_all", bufs=1, name=R + "cos_all")
        sinF_all = csp.tile([P, S], bf16, tag="sinF_all", bufs=1, name=R + "sinF_all")
        nc.scalar.dma_start(out=cos_all[:], in_=cosT[:])
        nc.scalar.dma_start(out=sinF_all[:], in_=sinFT[:])

        for s in range(N_QC):
            ssl = slice(s * QC, (s + 1) * QC)
            cos_sb = cos_all[:, ssl]
            sinF_sb = sinF_all[:, ssl]

            ps = [
                pps.tile([P, QC], f32, tag=f"proj{(f + s) % 7}", name=f"{R}proj{f}_{s}")
                for f in range(6)
            ]
            sc2 = s // 2
            for e in range(N_E):
                if (sc2, e) in x_pre:
                    x_r2 = x_pre[(sc2, e)]
                else:
                    x_r2 = xp.tile(
                        [P, 2 * QC], bf16, tag="x_r", bufs=18, name=f"{R}x_r{sc2}_{e}"
                    )
                    nc.scalar.dma_start(
                        out=x_r2[:],
                        in_=xT[e * P:(e + 1) * P, sc2 * 2 * QC:(sc2 + 1) * 2 * QC],
                    )
                    x_pre[(sc2, e)] = x_r2
                x_r = x_r2[:, (s % 2) * QC:(s % 2 + 1) * QC]
                for f in range(6):
                    nc.tensor.matmul(
                        ps[f][:],
                        wqkv_r[e][:, f * P:(f + 1) * P],
                        x_r,
                        start=(e == 0),
                        stop=(e == N_E - 1),
                    )

            # evict psum fast via ACT copy (frees the bank), then
            # RoPE on SBUF off the PSUM critical path:
            # dst = stage*cos + shift(stage)*sinF
            def rope_evict(dst, psrc, tmp_name):
                stage = xp.tile([P, QC], bf16, tag="rstage", bufs=4, name="st" + tmp_name)
                nc.scalar.copy(stage[:], psrc)
                # partition-rotate by 64 via single-input copies
                # (SBUF TT requires equal base partitions on HW)
                shf = xp.tile([P, QC], bf16, tag="rope_shf", name="sh" + tmp_name)
                H = P // 2
                nc.vector.tensor_copy(shf[0:H, :], stage[H:P, :])
                nc.vector.tensor_copy(shf[H:P, :], stage[0:H, :])
                nc.vector.tensor_mul(shf[:], shf[:], sinF_sb)
                nc.vector.tensor_mul(stage[:], stage[:], cos_sb)
                nc.vector.tensor_add(dst, stage[:], shf[:])

            rope_evict(kT_r[:, ssl], ps[4][:], f"{R}rope_k{s}")

            # v: evict bf16, then PE-transpose each 128 block
            v_sb = vsp.tile([P, QC], bf16, tag="v_sb", name=f"{R}v_sb{s}")
            nc.scalar.copy(v_sb[:], ps[5][:])
            for j in range(QC // P):
                kt = s * (QC // P) + j
                tps = vtps.tile([P, P], bf16, tag="vtr", name=f"{R}vtr{kt}")
                nc.tensor.transpose(tps[:], v_sb[:, j * P:(j + 1) * P], ident[:])
                nc.vector.tensor_copy(v_nat[:, kt * P:(kt + 1) * P], tps[:])

            for h in range(4):
                rope_evict(qT_r[h][:, ssl], ps[h][:], f"{R}rope_q{h}_{s}")

    # ---------------- Phase 2+3: attention + out-projection
    with (
        tc.tile_pool(name=R + "wo_pool", bufs=1) as wop,
        tc.tile_pool(name=R + "exp_pool", bufs=6) as ep,
        tc.tile_pool(name=R + "outT_pool", bufs=1) as op_,
        tc.tile_pool(name=R + "small_pool", bufs=3) as sp,
        tc.tile_pool(name=R + "sc_ps", bufs=2, space="PSUM") as scp,
        tc.tile_pool(name=R + "pv_ps", bufs=1, space="PSUM") as pvp,
        tc.tile_pool(name=R + "denbc_ps", bufs=1, space="PSUM") as dbp,
        tc.tile_pool(name=R + "yp_ps", bufs=2, space="PSUM") as ypp,
    ):
        # Wo resident load (needed first by oproj(qc0), after attn(qc0))
        wo_r = []
        for d in range(4):
            t = wop.tile([P, E], bf16, tag=f"wo_r{d}", name=f"{R}wo_r{d}")
            nc.sync.dma_start(out=t[:], in_=woT[d * P:(d + 1) * P, :])
            wo_r.append(t)

        outT = [
            op_.tile([P, S], bf16, tag=f"outT{h}", name=f"{R}outT{h}")
            for h in range(4)
        ]

        for qc in range(N_QC):
            qsl = slice(qc * QC, (qc + 1) * QC)
            full_kts, units = _full_partial(qc)
            for h in range(4):
                pv = pvp.tile([P, QC], f32, tag="pv", name=f"{R}pv{qc}_{h}")
                den = dbp.tile([1, QC], f32, tag="denbc", name=f"{R}den{qc}_{h}")

                # PSUM accumulate flags: start=True on the first
                # matmul into the bank zeroes the whole 2KB zero
                # region, so later matmuls accumulate start=False
                # into either q-half; stop=True only on the last.
                ops = []  # (kind, payload)
                for i in range(0, len(full_kts), 2):
                    ops.append(("full_pair", full_kts[i:i + 2]))
                for i in range(0, len(units), 4):
                    ops.append(("unit_quad", units[i:i + 4]))
                n_acc = sum(len(pl) for _, pl in ops)

                def acc_flags(oid_):
                    return oid_ == 0, oid_ == n_acc - 1

                oid = 0
                for kind, pl in ops:
                    if kind == "full_pair":
                        pair = pl
                        w = QC
                        sc = scp.tile(
                            [P, 2 * QC], f32, tag="sc",
                            name=f"{R}sc{qc}_{h}_{pair[0]}",
                        )
                        for j, kt in enumerate(pair):
                            nc.tensor.matmul(
                                sc[:, j * w:(j + 1) * w],
                                kT_r[:, kt * P:(kt + 1) * P],
                                qT_r[h][:, qsl],
                                start=True,
                                stop=True,
                            )
                        ex = ep.tile(
                            [P, 2 * QC], bf16, tag="ex",
                            name=f"{R}ex{qc}_{h}_f{pair[0]}",
                        )
                        nc.scalar.activation(
                            ex[:, : len(pair) * w],
                            sc[:, : len(pair) * w],
                            Exp,
                            scale=SCALE,
                        )
                        for j, kt in enumerate(pair):
                            exj = ex[:, j * w:(j + 1) * w]
                            st, sp_ = acc_flags(oid)
                            nc.tensor.matmul(
                                pv[:],
                                v_nat[:, kt * P:(kt + 1) * P],
                                exj,
                                start=st,
                                stop=sp_,
                            )
                            nc.tensor.matmul(
                                den[:],
                                ones_col[:],
                                exj,
                                start=st,
                                stop=sp_,
                            )
                            oid += 1
                    else:
                        upair = pl
                        w = HC
                        sc = scp.tile(
                            [P, 2 * QC], f32, tag="sc",
                            name=f"{R}scu{qc}_{h}_{upair[0][0]}_{upair[0][1]}",
                        )  # holds up to 4 x 256-wide units
                        for j, (kt, h2, dh) in enumerate(upair):
                            q0 = qc * QC + h2 * HC
                            nc.tensor.matmul(
                                sc[:, j * w:(j + 1) * w],
                                kT_r[:, kt * P:(kt + 1) * P],
                                qT_r[h][:, q0:q0 + HC],
                                start=True,
                                stop=True,
                            )
                        ex = ep.tile(
                            [P, 2 * QC], bf16, tag="ex",
                            name=f"{R}exu{qc}_{h}_{upair[0][0]}_{upair[0][1]}",
                        )
                        nc.scalar.activation(
                            ex[:, : len(upair) * w],
                            sc[:, : len(upair) * w],
                            Exp,
                            scale=SCALE,
                        )
                        for j, (kt, h2, dh) in enumerate(upair):
                            exj = ex[:, j * w:(j + 1) * w]
                            if dh is not None:
                                # zero out-of-window (iota = dh + q - k):
                                # dh <= 0 cuts dist >= 0; dh >= 640 cuts
                                # dist < WINDOW. exactly one cut per tile.
                                import concourse.mybir as _mybir
                                if dh <= 0:
                                    nc.gpsimd.affine_select(
                                        out=exj, in_=exj,
                                        pattern=[[1, HC]],
                                        compare_op=_mybir.AluOpType.is_ge,
                                        fill=0.0, base=dh,
                                        channel_multiplier=-1,
                                    )
                                else:
                                    # keep where (W-1-dh) + k - q >= 0
                                    nc.gpsimd.affine_select(
                                        out=exj, in_=exj,
                                        pattern=[[-1, HC]],
                                        compare_op=_mybir.AluOpType.is_ge,
                                        fill=0.0, base=(WINDOW - 1) - dh,
                                        channel_multiplier=1,
                                    )
                            st, sp_ = acc_flags(oid)
                            pv_reg = pv[:, h2 * HC:(h2 + 1) * HC]
                            den_reg = den[:, h2 * HC:(h2 + 1) * HC]
                            nc.tensor.matmul(
                                pv_reg,
                                v_nat[:, kt * P:(kt + 1) * P],
                                exj,
                                start=st,
                                stop=sp_,
                            )
                            nc.tensor.matmul(
                                den_reg,
                                ones_col[:],
                                exj,
                                start=st,
                                stop=sp_,
                            )
                            oid += 1

                # normalize: outT[h][:, qsl] = pv * (1/den) broadcast
                recip = sp.tile([1, QC], f32, tag="recip", name=f"{R}rc{qc}_{h}")
                nc.vector.reciprocal(recip[:], den[:])
                bc_sb = sp.tile([P, QC], f32, tag="bc_sb", name=f"{R}bcs{qc}_{h}")
                nc.gpsimd.partition_broadcast(bc_sb[:], recip[:])
                nc.vector.tensor_mul(outT[h][:, qsl], pv[:], bc_sb[:])

            # out-projection for this q chunk (own psum pool)
            for e in range(N_E):
                yp = ypp.tile([P, QC], f32, tag="yp", name=f"{R}yp{qc}_{e}")
                for d in range(4):
                    nc.tensor.matmul(
                        yp[:],
                        wo_r[d][:, e * P:(e + 1) * P],
                        outT[d][:, qsl],
                        start=(d == 0),
                        stop=(d == 3),
                    )
                y_sb = sp.tile([P, QC], bf16, tag="y_sb", bufs=6, name=f"{R}ysb{qc}_{e}")
                nc.vector.tensor_copy(y_sb[:], yp[:])
                nc.sync.dma_start(out=y[e * P:(e + 1) * P, qsl], in_=y_sb[:])


def _bf16(a):
    import ml_dtypes

    return np.ascontiguousarray(a).astype(ml_dtypes.bfloat16)


def make_in_maps(x, cos, sin, Wq, Wk, Wv, Wo):
    cosT = np.ascontiguousarray(cos[:, 0, :].T)  # [128, S]
    sinT = sin[:, 0, :].T
    sinFT = np.concatenate([-sinT[: HD // 2], sinT[HD // 2:]], axis=0)
    sinFT = np.ascontiguousarray(sinFT.astype(np.float32))
    cosT_b = _bf16(cosT)
    sinFT_b = _bf16(sinFT)
    in_maps = []
    for c in range(8):
        b, g = c // 4, c % 4
        wq_g = Wq[g * 4 * HD:(g + 1) * 4 * HD, :]  # [512, E]
        wk_g = Wk[g * HD:(g + 1) * HD, :]  # [128, E]
        wv_g = Wv[g * HD:(g + 1) * HD, :]
        wqkvT = np.ascontiguousarray(
            np.concatenate([wq_g, wk_g, wv_g], axis=0).T
        )  # [E, 768]
        woT_g = np.ascontiguousarray(Wo[:, g * 4 * HD:(g + 1) * 4 * HD].T)  # [512, E]
        in_maps.append(
            {
                "xT": _bf16(x[b].T),
                "wqkvT": _bf16(wqkvT),
                "woT": _bf16(woT_g),
                "cosT": cosT_b,
                "sinFT": sinFT_b,
            }
        )
    return in_maps


_NC_CACHE = {}


def get_nc():
    if "nc" not in _NC_CACHE:
        _NC_CACHE["nc"] = build_nc()
    return _NC_CACHE["nc"]


def kernel(x, cos, sin, Wq, Wk, Wv, Wo):
    from concourse.bass_utils import run_bass_kernel_spmd

    x = np.asarray(x, dtype=np.float32)
    cos = np.asarray(cos, dtype=np.float32)
    sin = np.asarray(sin, dtype=np.float32)
    Wq = np.asarray(Wq, dtype=np.float32)
    Wk = np.asarray(Wk, dtype=np.float32)
    Wv = np.asarray(Wv, dtype=np.float32)
    Wo = np.asarray(Wo, dtype=np.float32)

    nc = get_nc()
    in_maps = make_in_maps(x, cos, sin, Wq, Wk, Wv, Wo)
    res = run_bass_kernel_spmd(nc, in_maps, core_ids=list(range(8)))
    out = np.zeros((B, S, E), dtype=np.float32)
    for c in range(8):
        b = c // 4
        out[b] += res.results[c]["y"].T.astype(np.float32)
    return out
